# revision 3
# baseline (speedup 1.0000x reference)
"""AttentionCNN distributed Trainium2 kernel (8 NeuronCores).

Strategy:
  Phase 1 (channel-sharded): core i owns channels [8i, 8i+8). Per channel j:
    enc^T_c = relu(W_c^T-contract-x) computed e-major via TensorE, K=S tiles.
    Epilogue writes bf16 enc^T into AllToAll bounce buffers, laid out
    [dst_rank, e, j%4, b_local] so the A2A converts channel-sharding into
    batch-sharding with channels landing in global order.
  Phase 2: two AllToAlls (channels j=0..3 and j=4..7) so the first one
    overlaps the second half of the encoder.
  Phase 3 (batch-sharded): core i owns batches [32i, 32i+32).
    q^T/k^T batched GEMMs (bias via ScalarE), v per-batch in (c, a) layout,
    scores = q·k^T per batch, softmax without max-subtraction (scores are
    bounded ~3), conv folded through the softmax normalization:
      conv[b,:] = relu(sum_d (sum_c conv_w[c]/ssum[c,b] * e^{s[c,d]}) v[b,d,:] + cb)
    then the final 2-class FC, all on-chip.

Compute dtype bf16 (f32 PSUM accumulation), rel-err ~1e-2 vs f32 reference.
"""
import sys

if "/opt/trn_rl_repo" not in sys.path:
    sys.path.insert(0, "/opt/trn_rl_repo")

import numpy as np
import ml_dtypes

from concourse import bass, bacc, tile, mybir
from concourse import bass_utils

BF = mybir.dt.bfloat16
F32 = mybir.dt.float32
BF_NP = ml_dtypes.bfloat16

B, S_FULL, C, E, A, NCLS = 256, 3000, 64, 512, 256, 2
W = 8
CL = C // W          # channels per core
BL = B // W          # batches per core
EH = E // 128        # 4 e-tiles
AH = A // 128        # 2 a-tiles
NG = BL // 8         # batch groups of 8


def build(S=S_FULL):
    KT = (S + 127) // 128
    kfull = S // 128
    krem = S - kfull * 128

    nc = bacc.Bacc("TRN2", target_bir_lowering=False, debug=False, num_devices=W)

    xT = nc.dram_tensor("xT", [CL, S, B], BF, kind="ExternalInput")
    wT = nc.dram_tensor("wT", [CL, S, E], BF, kind="ExternalInput")
    encb = nc.dram_tensor("encb", [CL, E], F32, kind="ExternalInput")
    wqT = nc.dram_tensor("wqT", [E, A], BF, kind="ExternalInput")
    wkT = nc.dram_tensor("wkT", [E, A], BF, kind="ExternalInput")
    wvT = nc.dram_tensor("wvT", [E, A], BF, kind="ExternalInput")
    bq = nc.dram_tensor("bq", [AH, 128], F32, kind="ExternalInput")
    bk = nc.dram_tensor("bk", [AH, 128], F32, kind="ExternalInput")
    bv = nc.dram_tensor("bv", [C, A], F32, kind="ExternalInput")
    convw = nc.dram_tensor("convw", [C, 1], F32, kind="ExternalInput")
    convb = nc.dram_tensor("convb", [128, 1], F32, kind="ExternalInput")
    fcwT = nc.dram_tensor("fcwT", [A, NCLS], BF, kind="ExternalInput")
    fcb = nc.dram_tensor("fcb", [NCLS, 1], F32, kind="ExternalInput")
    out = nc.dram_tensor("out", [NCLS, BL], F32, kind="ExternalOutput")

    RG = [list(range(W))]
    Relu = mybir.ActivationFunctionType.Relu
    Iden = mybir.ActivationFunctionType.Identity
    Exp = mybir.ActivationFunctionType.Exp

    with tile.TileContext(nc) as tc:
        with tc.tile_pool(name="dram", bufs=1, space="DRAM") as dpool, \
             tc.tile_pool(name="sb", bufs=1) as sb, \
             tc.tile_pool(name="stream", bufs=2) as stream, \
             tc.tile_pool(name="pp", bufs=1, space="PSUM") as pp:

            ain0 = dpool.tile([W, E, 4, BL], BF, name="ain0", tag="ain0")
            ain1 = dpool.tile([W, E, 4, BL], BF, name="ain1", tag="ain1")
            aout0 = dpool.tile([W, E, 4, BL], BF, name="aout0", tag="aout0")
            aout1 = dpool.tile([W, E, 4, BL], BF, name="aout1", tag="aout1")
            ain = (ain0, ain1)
            aout = (aout0, aout1)

            # --- persistent small tensors -------------------------------
            encb_sb = sb.tile([128, CL * EH], F32, name="encb_sb", tag="encb_sb")
            nc.sync.dma_start(
                encb_sb[:].rearrange("p (j eh) -> p j eh", j=CL),
                encb[:, :].rearrange("j (eh el) -> el j eh", el=128),
            )
            wq_sb = sb.tile([128, EH * A], BF, name="wq_sb", tag="wq_sb")
            wk_sb = sb.tile([128, EH * A], BF, name="wk_sb", tag="wk_sb")
            wv_sb = sb.tile([128, EH * A], BF, name="wv_sb", tag="wv_sb")
            for wsb, wdr in ((wq_sb, wqT), (wk_sb, wkT), (wv_sb, wvT)):
                nc.scalar.dma_start(
                    wsb[:].rearrange("p (eh a) -> p eh a", eh=EH),
                    wdr[:, :].rearrange("(eh el) a -> el eh a", el=128),
                )
            bq_sb = sb.tile([128, AH], F32, name="bq_sb", tag="bq_sb")
            bk_sb = sb.tile([128, AH], F32, name="bk_sb", tag="bk_sb")
            nc.scalar.dma_start(bq_sb[:], bq[:, :].rearrange("ah p -> p ah"))
            nc.scalar.dma_start(bk_sb[:], bk[:, :].rearrange("ah p -> p ah"))
            bv_sb = sb.tile([C, A], F32, name="bv_sb", tag="bv_sb")
            nc.scalar.dma_start(bv_sb[:], bv[:, :])
            convw_sb = sb.tile([C, 1], F32, name="convw_sb", tag="convw_sb")
            nc.scalar.dma_start(convw_sb[:], convw[:, :])
            convb_sb = sb.tile([128, 1], F32, name="convb_sb", tag="convb_sb")
            nc.scalar.dma_start(convb_sb[:], convb[:, :])
            fcw_sb = sb.tile([128, AH * NCLS], BF, name="fcw_sb", tag="fcw_sb")
            nc.scalar.dma_start(
                fcw_sb[:].rearrange("p (ah k) -> p ah k", ah=AH),
                fcwT[:, :].rearrange("(ah al) k -> al ah k", al=128),
            )
            fcb_sb = sb.tile([NCLS, 1], F32, name="fcb_sb", tag="fcb_sb")
            nc.scalar.dma_start(fcb_sb[:], fcb[:, :])

            # --- phase 1: encoder (channel-sharded) ---------------------
            for j in range(CL):
                xj = stream.tile([128, KT * B], BF, name="xj", tag="xj")
                wj = stream.tile([128, KT * E], BF, name="wj", tag="wj")
                xv = xj[:].rearrange("p (k b) -> p k b", k=KT)
                wv = wj[:].rearrange("p (k e) -> p k e", k=KT)
                nc.sync.dma_start(
                    xv[:, :kfull, :],
                    xT[j, : kfull * 128, :].rearrange("(k sl) b -> sl k b", sl=128),
                )
                nc.sync.dma_start(
                    wv[:, :kfull, :],
                    wT[j, : kfull * 128, :].rearrange("(k sl) e -> sl k e", sl=128),
                )
                if krem:
                    nc.sync.dma_start(xv[:krem, kfull, :], xT[j, kfull * 128 :, :])
                    nc.sync.dma_start(wv[:krem, kfull, :], wT[j, kfull * 128 :, :])

                h, jj = divmod(j, 4)
                for et in range(EH):
                    ps = pp.tile([128, B], F32, name="encps", tag="encps", bufs=2)
                    for kt in range(KT):
                        kn = min(128, S - kt * 128)
                        nc.tensor.matmul(
                            ps[:, :],
                            lhsT=wv[:kn, kt, et * 128 : (et + 1) * 128],
                            rhs=xv[:kn, kt, :],
                            start=(kt == 0),
                            stop=(kt == KT - 1),
                        )
                    enc_sb = stream.tile([128, B], BF, name="enc_sb", tag="enc_sb", bufs=3)
                    nc.scalar.activation(
                        enc_sb[:], ps[:, :], Relu,
                        bias=encb_sb[:, j * EH + et : j * EH + et + 1],
                    )
                    nc.sync.dma_start(
                        ain[h][:].rearrange("r e j b -> e r j b")[
                            et * 128 : (et + 1) * 128, :, jj, :
                        ],
                        enc_sb[:].rearrange("p (r b) -> p r b", r=W),
                    )
                if j == CL // 2 - 1:
                    nc.gpsimd.collective_compute(
                        "AllToAll", mybir.AluOpType.bypass, replica_groups=RG,
                        ins=[ain0.opt()], outs=[aout0.opt()],
                    )
                if j == CL - 1:
                    nc.gpsimd.collective_compute(
                        "AllToAll", mybir.AluOpType.bypass, replica_groups=RG,
                        ins=[ain1.opt()], outs=[aout1.opt()],
                    )

            # --- phase 3 load: assemble enc (batch-sharded, e-major) ----
            enc_all = sb.tile([128, EH * C * BL], BF, name="enc_all", tag="enc_all")
            ev = enc_all[:].rearrange("p (eh c b) -> p eh c b", eh=EH, c=C)
            for h in range(2):
                for r in range(W):
                    c0 = 8 * r + 4 * h
                    nc.scalar.dma_start(
                        ev[:, :, c0 : c0 + 4, :],
                        aout[h][:].rearrange("r (eh el) j b -> r el eh j b", el=128)[r],
                    )

            # --- phase 3a: q^T / k^T (batched), v (per-batch) -----------
            q_sb = sb.tile([128, AH * C * BL], BF, name="q_sb", tag="q_sb")
            k_sb = sb.tile([128, AH * C * BL], BF, name="k_sb", tag="k_sb")
            qv4 = q_sb[:].rearrange(
                "p (ah rr h2 jj b) -> p ah h2 rr jj b", ah=AH, rr=8, h2=2, jj=4
            )
            kv4 = k_sb[:].rearrange(
                "p (ah rr h2 jj b) -> p ah h2 rr jj b", ah=AH, rr=8, h2=2, jj=4
            )
            ev6 = enc_all[:].rearrange(
                "p (eh rr h2 jj b) -> p eh h2 rr jj b", eh=EH, rr=8, h2=2, jj=4
            )
            for h in range(2):
                for rh in range(2):
                    for wsb, bsb, dstv in ((wq_sb, bq_sb, qv4), (wk_sb, bk_sb, kv4)):
                        wvr = wsb[:].rearrange("p (eh a) -> p eh a", eh=EH)
                        for ah in range(AH):
                            ps = pp.tile([128, 512], F32, name="qkps", tag="qkps", bufs=2)
                            for eh in range(EH):
                                nc.tensor.matmul(
                                    ps[:, :],
                                    lhsT=wvr[:, eh, ah * 128 : (ah + 1) * 128],
                                    rhs=ev6[:, eh, h, rh * 4 : (rh + 1) * 4, :, :],
                                    start=(eh == 0),
                                    stop=(eh == EH - 1),
                                )
                            nc.scalar.activation(
                                dstv[:, ah, h, rh * 4 : (rh + 1) * 4, :, :],
                                ps[:, :].rearrange("p (rr jj b) -> p rr jj b", rr=4, jj=4),
                                Iden, bias=bsb[:, ah : ah + 1],
                            )

            v_sb = sb.tile([C, BL * A], BF, name="v_sb", tag="v_sb")
            vv = v_sb[:].rearrange("p (b a) -> p b a", b=BL)
            wvv = wv_sb[:].rearrange("p (eh a) -> p eh a", eh=EH)
            ev4 = enc_all[:].rearrange("p (eh c b) -> p eh c b", eh=EH, c=C)
            for b in range(BL):
                psv = pp.tile([C, A], F32, name="vps", tag="qkps", bufs=2)
                for eh in range(EH):
                    nc.tensor.matmul(
                        psv[:, :], lhsT=ev4[:, eh, :, b], rhs=wvv[:, eh],
                        start=(eh == 0), stop=(eh == EH - 1),
                    )
                nc.vector.tensor_add(vv[:, b, :], psv[:, :], bv_sb[:])

            # --- phase 3b: scores / softmax / conv-fold / fc ------------
            probs_sb = sb.tile([C, BL * C], BF, name="probs_sb", tag="probs_sb")
            pv = probs_sb[:].rearrange("p (b d) -> p b d", b=BL)
            rsum = sb.tile([C, BL], F32, name="rsum", tag="rsum")
            rinv = sb.tile([C, BL], F32, name="rinv", tag="rinv")
            wprime = sb.tile([C, BL], BF, name="wprime", tag="wprime")
            wp_sb = sb.tile([C, BL], BF, name="wp_sb", tag="wp_sb")
            convT = sb.tile([128, 2 * BL], BF, name="convT", tag="convT")
            cvv = convT[:].rearrange("p (ah b) -> p ah b", ah=2)
            qv3 = q_sb[:].rearrange("p (ah c b) -> p ah c b", ah=AH, c=C)
            kv3 = k_sb[:].rearrange("p (ah c b) -> p ah c b", ah=AH, c=C)
            for g in range(NG):
                psc = pp.tile([C, 8 * C], F32, name="scps", tag="qkps", bufs=2)
                scv = psc[:, :].rearrange("p (bb d) -> p bb d", bb=8)
                for bb in range(8):
                    b = g * 8 + bb
                    for ah in range(AH):
                        nc.tensor.matmul(
                            scv[:, bb, :],
                            lhsT=qv3[:, ah, :, b], rhs=kv3[:, ah, :, b],
                            start=(ah == 0), stop=(ah == AH - 1),
                        )
                nc.scalar.activation(pv[:, g * 8 : (g + 1) * 8, :], scv[:, :, :], Exp)
                nc.vector.reduce_sum(
                    rsum[:, g * 8 : (g + 1) * 8], pv[:, g * 8 : (g + 1) * 8, :],
                    axis=mybir.AxisListType.X,
                )
                nc.vector.reciprocal(
                    rinv[:, g * 8 : (g + 1) * 8], rsum[:, g * 8 : (g + 1) * 8]
                )
                nc.vector.tensor_scalar_mul(
                    wprime[:, g * 8 : (g + 1) * 8],
                    rinv[:, g * 8 : (g + 1) * 8], convw_sb[:],
                )
                psw = pp.tile([C, 8], F32, name="wpps", tag="smallps", bufs=2)
                for bb in range(8):
                    b = g * 8 + bb
                    nc.tensor.matmul(
                        psw[:, bb : bb + 1],
                        lhsT=pv[:, b, :], rhs=wprime[:, b : b + 1],
                        start=True, stop=True,
                    )
                nc.vector.tensor_copy(wp_sb[:, g * 8 : (g + 1) * 8], psw[:, :])
                for ah in range(2):
                    psco = pp.tile([128, 8], F32, name="psco", tag="smallps", bufs=2)
                    for bb in range(8):
                        b = g * 8 + bb
                        nc.tensor.matmul(
                            psco[:, bb : bb + 1],
                            lhsT=vv[:, b, ah * 128 : (ah + 1) * 128],
                            rhs=wp_sb[:, b : b + 1],
                            start=True, stop=True,
                        )
                    nc.scalar.activation(
                        cvv[:, ah, g * 8 : (g + 1) * 8], psco[:, :],
                        Relu, bias=convb_sb[:],
                    )
            fcv = fcw_sb[:].rearrange("p (ah k) -> p ah k", ah=AH)
            psf = pp.tile([NCLS, BL], F32, name="psf", tag="smallps", bufs=2)
            for ah in range(AH):
                nc.tensor.matmul(
                    psf[:, :], lhsT=fcv[:, ah, :], rhs=cvv[:, ah, :],
                    start=(ah == 0), stop=(ah == AH - 1),
                )
            outT = sb.tile([NCLS, BL], F32, name="outT", tag="outT")
            nc.scalar.activation(outT[:], psf[:, :], Iden, bias=fcb_sb[:])
            nc.sync.dma_start(out[:, :], outT[:])

    nc.compile()
    return nc


def make_in_maps(inputs, S=S_FULL):
    """Shard + layout-transform the full inputs for the 8 cores."""
    x = np.asarray(inputs["x"])[:, :S, :]
    enc_W = np.asarray(inputs["enc_W"])[:, :, :S]
    enc_b = np.asarray(inputs["enc_b"], dtype=np.float32)
    sc = np.float32(1.0 / np.sqrt(A))

    xT_all = x.transpose(2, 1, 0).astype(BF_NP)           # (C, S, B)
    wT_all = enc_W.transpose(0, 2, 1).astype(BF_NP)       # (C, S, E)
    wqT = (np.asarray(inputs["Wq"]).T * sc).astype(BF_NP)  # (E, A), prescaled
    wkT = np.asarray(inputs["Wk"]).T.astype(BF_NP)
    wvT = np.asarray(inputs["Wv"]).T.astype(BF_NP)
    bq = (np.asarray(inputs["bq"], dtype=np.float32) * sc).reshape(AH, 128)
    bk = np.asarray(inputs["bk"], dtype=np.float32).reshape(AH, 128)
    bv = np.ascontiguousarray(
        np.broadcast_to(np.asarray(inputs["bv"], dtype=np.float32), (C, A))
    )
    convw = np.asarray(inputs["conv_w"], dtype=np.float32).reshape(C, 1)
    convb = np.full((128, 1), np.asarray(inputs["conv_b"], dtype=np.float32)[0],
                    dtype=np.float32)
    fcwT = np.asarray(inputs["fc_w"]).T.astype(BF_NP)      # (A, NCLS)
    fcb = np.asarray(inputs["fc_b"], dtype=np.float32).reshape(NCLS, 1)

    shared = dict(wqT=wqT, wkT=wkT, wvT=wvT, bq=bq, bk=bk, bv=bv,
                  convw=convw, convb=convb, fcwT=fcwT, fcb=fcb)
    in_maps = []
    for i in range(W):
        m = dict(shared)
        m["xT"] = np.ascontiguousarray(xT_all[i * CL : (i + 1) * CL])
        m["wT"] = np.ascontiguousarray(wT_all[i * CL : (i + 1) * CL])
        m["encb"] = np.ascontiguousarray(enc_b[i * CL : (i + 1) * CL])
        in_maps.append(m)
    return in_maps


_CACHE = {}


def _get_nc(S=S_FULL):
    if S not in _CACHE:
        _CACHE[S] = build(S)
    return _CACHE[S]


def run(inputs, S=S_FULL, **run_kwargs):
    nc = _get_nc(S)
    in_maps = make_in_maps(inputs, S)
    res = bass_utils.run_bass_kernel_spmd(
        nc, in_maps, core_ids=list(range(W)), **run_kwargs
    )
    outs = [res.results[i]["out"] for i in range(W)]      # each (2, BL)
    full = np.concatenate(outs, axis=1).T                  # (B, 2)
    return np.ascontiguousarray(full, dtype=np.float32), res


def kernel(**inputs):
    out, _ = run(inputs)
    return out
